# revision 10
# baseline (speedup 1.0000x reference)
"""AudioAttention forward on 8 Trainium2 NeuronCores (Bass/Tile).

Reference computation (eval-mode AudioAttention):
    z      = mean_pool(Z_img)                    # [B, C]
    z_img, query = z[:, :C-A], z[:, C-A:]
    snd    = Z_snd[pad_idx]                      # [G, S, C] ragged gather
    value, key = snd[..., :C-A], snd[..., C-A:]
    scores = query @ key^T  (per group), masked softmax over S
    M_snd  = attn @ value                        # [G, B, C-A]
    M_img  = broadcast(z_img)                    # [G, B, C-A]

Sharding: groups are sorted by size and dealt round-robin to the 8
cores, so every core gets the same per-slot capacity profile -> one
SPMD program serves all cores (only the DRAM contents differ).
Capacities are rounded up to multiples of 128 so every 128-token chunk
is partition-aligned, which lets the whole key/value arrays live in
SBUF and stream in via a handful of large DMAs (the DMA *instruction*
dispatch on the queue engine costs ~0.8us each, so instruction count
matters more than bytes).

Device kernel, per 128-token chunk k of group slot j:
  scoresT [128,B] = matmul(lhsT=keyT_ext[65, 128], rhs=qT_ext[65, B])
      where row 64 of keyT_ext carries (-shift) for valid tokens and
      -30000 for padding, and row 64 of qT_ext is ones -> the mask and
      the softmax shift are folded into the contraction for free
      (exp(-30000) == 0 exactly, so pad tokens vanish).
  attnT = exp(scoresT)              (ACT engine, PSUM -> SBUF)
  m_j [B, 450] += matmul(lhsT=attnT[128, B], rhs=val_ext[128, 450])
      where val_ext column 448 is 1.0 for valid rows -> column 448
      accumulates the softmax denominator (449 is zero padding: the
      fp32r matmul requires an even destination free size).
  out_j = m_j[:, :448] * reciprocal(m_j[:, 448])  (DVE), one final DMA.

Matmuls run as float32r (TF32): same bits as fp32, 4x the fp32 PE
streaming rate. The softmax shift per group is a Cauchy-Schwarz upper
bound on the scores (max_b |q_b| * max_s |k_s|), so exp never
overflows; softmax is shift-invariant so the result is exact.
"""

import sys

if "/opt/trn_rl_repo" not in sys.path:
    sys.path.insert(0, "/opt/trn_rl_repo")

import numpy as np

N_CORES = 8
CHUNK = 128
N_SPLIT_V = 8  # value-array DMA split (parallel queues + early compute start)
N_SPLIT_K = 4  # key-array DMA split

LAST_RESULTS = None  # BassKernelResults of the most recent run (for test harness)


def _build_program(caps, gpc, ca):
    """One Bass program shared by all 8 cores.

    caps: per-slot token capacities, multiples of 128 (same on every core).
    gpc:  groups (slots) per core.
    ca:   C - A (value feature width).
    """
    from concourse import bacc, mybir
    from concourse.tile import TileContext

    vw = ca + 2  # value row width: features + denominator column + pad
    n_chunks = int(sum(caps)) // CHUNK
    sum_caps = n_chunks * CHUNK
    nc = bacc.Bacc(None, target_bir_lowering=False, debug=False)

    f32 = mybir.dt.float32
    f32r = mybir.dt.float32r
    keys_d = nc.dram_tensor("keysT", [65, sum_caps], f32r, kind="ExternalInput")
    vals_d = nc.dram_tensor("vals", [sum_caps, vw], f32r, kind="ExternalInput")
    qt_d = nc.dram_tensor("qT", [65, 16], f32r, kind="ExternalInput")
    out_d = nc.dram_tensor("out", [gpc, 16, ca], f32, kind="ExternalOutput")

    # chunk-aligned rows -> [partition, chunk, col] view for a few big DMAs
    vals_v = vals_d.rearrange("(k p) c -> p k c", p=CHUNK)

    def split(n, parts):
        q, r = divmod(n, parts)
        out, a = [], 0
        for i in range(parts):
            b = a + q + (1 if i < r else 0)
            if b > a:
                out.append((a, b))
            a = b
        return out

    with TileContext(nc) as tc:
        with (
            tc.tile_pool(name="resid", bufs=1) as rpool,
            tc.tile_pool(name="attn", bufs=3) as apool,
            tc.tile_pool(name="recp", bufs=4) as recpool,
            tc.tile_pool(name="scps", bufs=2, space="PSUM") as scpsum,
            tc.tile_pool(name="mps", bufs=4, space="PSUM") as mpsum,
        ):
            qt = rpool.tile([65, 16], f32r)
            nc.sync.dma_start(out=qt[:], in_=qt_d[:])
            ktile = rpool.tile([65, sum_caps], f32r)
            vtile = rpool.tile([CHUNK, n_chunks * vw], f32r)
            vtile_v = vtile.rearrange("p (k c) -> p k c", c=vw)
            # Interleave key/value loads in consumption order, spread across
            # queue engines so the transfers ride parallel DMA channels.
            dma_engines = [nc.sync, nc.gpsimd]
            kparts = split(n_chunks, N_SPLIT_K)
            vparts = split(n_chunks, N_SPLIT_V)
            ei = 0
            for i in range(max(len(kparts), len(vparts))):
                if i < len(kparts):
                    a, b = kparts[i]
                    dma_engines[ei % len(dma_engines)].dma_start(
                        out=ktile[:, a * CHUNK : b * CHUNK],
                        in_=keys_d[:, a * CHUNK : b * CHUNK],
                    )
                    ei += 1
                if i < len(vparts):
                    a, b = vparts[i]
                    dma_engines[ei % len(dma_engines)].dma_start(
                        out=vtile_v[:, a:b, :], in_=vals_v[:, a:b, :]
                    )
                    ei += 1
            obuf = rpool.tile([16, gpc * ca], f32)

            # Scores + exp are per-token, so batch GEXP chunks (across slot
            # boundaries) into one PSUM bank and one ACT exp instruction.
            GEXP = 8
            attn_tiles = {}

            def ensure_attn(kk):
                gi = kk // GEXP
                if gi in attn_tiles:
                    return attn_tiles[gi]
                n = min(GEXP, n_chunks - gi * GEXP)
                sc = scpsum.tile([CHUNK, n * 16], f32, name=f"sc{gi}", tag="sc")
                for x in range(n):
                    t0 = (gi * GEXP + x) * CHUNK
                    nc.tensor.matmul(
                        sc[:, x * 16 : (x + 1) * 16],
                        ktile[:, t0 : t0 + CHUNK],
                        qt[:],
                        start=True,
                        stop=True,
                    )
                at = apool.tile([CHUNK, n * 16], f32r, name=f"at{gi}", tag="a")
                nc.scalar.activation(at[:], sc[:], mybir.ActivationFunctionType.Exp)
                attn_tiles[gi] = at
                return at

            k = 0
            for j in range(gpc):
                nck = int(caps[j]) // CHUNK
                m = mpsum.tile([16, vw], f32, name=f"m{j}", tag="m")
                for ci in range(nck):
                    kk = k + ci
                    at = ensure_attn(kk)
                    x = kk % GEXP
                    nc.tensor.matmul(
                        m[:],
                        at[:, x * 16 : (x + 1) * 16],
                        vtile[:, kk * vw : (kk + 1) * vw],
                        start=(ci == 0),
                        stop=(ci == nck - 1),
                    )
                rec = recpool.tile([16, 1], f32, name=f"r{j}", tag="r")
                nc.vector.reciprocal(rec[:], m[:, ca : ca + 1])
                nc.scalar.activation(
                    obuf[:, j * ca : (j + 1) * ca],
                    m[:, 0:ca],
                    mybir.ActivationFunctionType.Copy,
                    scale=rec[:],
                )
                k += nck

            out_v = out_d.rearrange("j b c -> b j c")
            obuf_v = obuf.rearrange("b (j c) -> b j c", c=ca)
            for i, (a, b) in enumerate(split(gpc, 4)):
                dma_engines[i % len(dma_engines)].dma_start(
                    out=out_v[:, a:b, :], in_=obuf_v[:, a:b, :]
                )

    nc.finalize()
    return nc


def kernel(Z_img, Z_snd, pad_idx, pad_mask, attn_dims):
    global LAST_RESULTS
    import os

    from concourse.bass_utils import run_bass_kernel_spmd

    Z_img = np.asarray(Z_img, dtype=np.float32)
    Z_snd = np.asarray(Z_snd, dtype=np.float32)
    pad_idx = np.asarray(pad_idx)
    pad_mask = np.asarray(pad_mask).astype(bool)
    A = int(attn_dims)

    B = Z_img.shape[0]
    C = Z_img.shape[1]
    CA = C - A
    G = pad_idx.shape[0]
    assert B == 16 and G % N_CORES == 0, (B, G)
    gpc = G // N_CORES

    z = Z_img.reshape(B, C, -1).mean(axis=2)
    z_img, query = z[:, :CA], z[:, CA:]

    sizes = pad_mask.sum(axis=1).astype(np.int64)
    order = np.argsort(-sizes, kind="stable")  # group ids, size descending
    caps = -(-np.maximum(sizes[order[0::N_CORES]], 1) // CHUNK) * CHUNK
    sum_caps = int(caps.sum())
    slot_off = np.concatenate([[0], np.cumsum(caps)[:-1]]).astype(np.int64)

    q_norm_max = float(np.linalg.norm(query, axis=1).max())

    # Per-core host-side layout prep.
    in_maps = []
    for c in range(N_CORES):
        keysT = np.zeros((65, sum_caps), dtype=np.float32)
        keysT[64, :] = -30000.0  # pad columns -> exp == 0 exactly
        vals = np.zeros((sum_caps, CA + 2), dtype=np.float32)
        for j in range(gpc):
            g = int(order[j * N_CORES + c])
            s = int(sizes[g])
            o = int(slot_off[j])
            if s == 0:
                # Reference yields NaN for empty groups (softmax of all
                # -inf); emit 0 instead via one fake zero-valued token.
                keysT[64, o] = 0.0
                vals[o, CA] = 1.0
                continue
            idx = pad_idx[g][pad_mask[g]]
            rows = Z_snd[idx]
            keysT[:64, o : o + s] = rows[:, CA:].T
            k_norm_max = float(np.linalg.norm(rows[:, CA:], axis=1).max())
            shift = min(q_norm_max * k_norm_max, 80.0)
            keysT[64, o : o + s] = -shift
            vals[o : o + s, :CA] = rows[:, :CA]
            vals[o : o + s, CA] = 1.0
        qT = np.empty((65, 16), dtype=np.float32)
        qT[:64] = query.T
        qT[64] = 1.0
        in_maps.append({"keysT": keysT, "vals": vals, "qT": qT})

    nc = _build_program(caps, gpc, CA)
    trace = bool(os.environ.get("AUDIOATTN_TRACE"))
    res = run_bass_kernel_spmd(
        nc, in_maps, list(range(N_CORES)), trace=trace,
        tmpdir=os.environ.get("AUDIOATTN_TRACE_DIR") if trace else None,
    )
    LAST_RESULTS = res

    M_snd = np.empty((G, B, CA), dtype=np.float32)
    for c in range(N_CORES):
        out_c = res.results[c]["out"]
        for j in range(gpc):
            M_snd[order[j * N_CORES + c]] = out_c[j]

    M_img = np.broadcast_to(z_img[None], (G, B, CA))
    return M_img, M_snd


# revision 11
# speedup vs baseline: 1.3235x; 1.3235x over previous
"""AudioAttention forward on 8 Trainium2 NeuronCores (Bass/Tile).

Reference computation (eval-mode AudioAttention):
    z      = mean_pool(Z_img)                    # [B, C]
    z_img, query = z[:, :C-A], z[:, C-A:]
    snd    = Z_snd[pad_idx]                      # [G, S, C] ragged gather
    value, key = snd[..., :C-A], snd[..., C-A:]
    scores = query @ key^T  (per group), masked softmax over S
    M_snd  = attn @ value                        # [G, B, C-A]
    M_img  = broadcast(z_img)                    # [G, B, C-A]

Sharding: groups are sorted by size and dealt round-robin to the 8
cores, so every core gets the same per-slot capacity profile -> one
SPMD program serves all cores (only the DRAM contents differ).
Capacities are rounded up to multiples of 128 so every 128-token chunk
is partition-aligned, which lets the whole key/value arrays live in
SBUF and stream in via a handful of large DMAs (the DMA *instruction*
dispatch on the queue engine costs ~0.8us each, so instruction count
matters more than bytes).

Device kernel, per 128-token chunk k of group slot j:
  scoresT [128,B] = matmul(lhsT=keyT_ext[65, 128], rhs=qT_ext[65, B])
      where row 64 of keyT_ext carries (-shift) for valid tokens and
      -30000 for padding, and row 64 of qT_ext is ones -> the mask and
      the softmax shift are folded into the contraction for free
      (exp(-30000) == 0 exactly, so pad tokens vanish).
  attnT = exp(scoresT)              (ACT engine, PSUM -> SBUF)
  m_j [B, 450] += matmul(lhsT=attnT[128, B], rhs=val_ext[128, 450])
      where val_ext column 448 is 1.0 for valid rows -> column 448
      accumulates the softmax denominator (449 is zero padding: the
      fp32r matmul requires an even destination free size).
  out_j = m_j[:, :448] * reciprocal(m_j[:, 448])  (DVE), one final DMA.

Matmuls run as float32r (TF32): same bits as fp32, 4x the fp32 PE
streaming rate. The softmax shift per group is a Cauchy-Schwarz upper
bound on the scores (max_b |q_b| * max_s |k_s|), so exp never
overflows; softmax is shift-invariant so the result is exact.
"""

import sys

if "/opt/trn_rl_repo" not in sys.path:
    sys.path.insert(0, "/opt/trn_rl_repo")

import numpy as np

N_CORES = 8
CHUNK = 128
N_SPLIT_V = 8  # value-array DMA split (parallel queues + early compute start)
N_SPLIT_K = 4  # key-array DMA split

LAST_RESULTS = None  # BassKernelResults of the most recent run (for test harness)


def _build_program(caps, gpc, ca):
    """One Bass program shared by all 8 cores.

    caps: per-slot token capacities, multiples of 128 (same on every core).
    gpc:  groups (slots) per core.
    ca:   C - A (value feature width).
    """
    from concourse import bacc, mybir
    from concourse.tile import TileContext

    vw = ca + 2  # value row width: features + denominator column + pad
    n_chunks = int(sum(caps)) // CHUNK
    sum_caps = n_chunks * CHUNK
    nc = bacc.Bacc(None, target_bir_lowering=False, debug=False)

    f32 = mybir.dt.float32
    f32r = mybir.dt.float32r
    f16 = mybir.dt.float16
    keys_d = nc.dram_tensor("keysT", [65, sum_caps], f32r, kind="ExternalInput")
    vals_d = nc.dram_tensor("vals", [CHUNK, n_chunks * vw], f16, kind="ExternalInput")
    qt_d = nc.dram_tensor("qT", [65, 16], f32r, kind="ExternalInput")
    out_d = nc.dram_tensor("out", [gpc, 16, ca], f32, kind="ExternalOutput")


    def split(n, parts):
        q, r = divmod(n, parts)
        out, a = [], 0
        for i in range(parts):
            b = a + q + (1 if i < r else 0)
            if b > a:
                out.append((a, b))
            a = b
        return out

    with TileContext(nc) as tc:
        with (
            tc.tile_pool(name="resid", bufs=1) as rpool,
            tc.tile_pool(name="attn", bufs=3) as apool,
            tc.tile_pool(name="recp", bufs=4) as recpool,
            tc.tile_pool(name="scps", bufs=2, space="PSUM") as scpsum,
            tc.tile_pool(name="mps", bufs=4, space="PSUM") as mpsum,
        ):
            qt = rpool.tile([65, 16], f32r)
            nc.sync.dma_start(out=qt[:], in_=qt_d[:])
            ktile = rpool.tile([65, sum_caps], f32r)
            vtile = rpool.tile([CHUNK, n_chunks * vw], f16)
            # Interleave key/value loads in consumption order, spread across
            # queue engines so the transfers ride parallel DMA channels.
            dma_engines = [nc.sync, nc.gpsimd]
            kparts = split(n_chunks, N_SPLIT_K)
            vparts = split(n_chunks, N_SPLIT_V)
            ei = 0
            for i in range(max(len(kparts), len(vparts))):
                if i < len(kparts):
                    a, b = kparts[i]
                    dma_engines[ei % len(dma_engines)].dma_start(
                        out=ktile[:, a * CHUNK : b * CHUNK],
                        in_=keys_d[:, a * CHUNK : b * CHUNK],
                    )
                    ei += 1
                if i < len(vparts):
                    a, b = vparts[i]
                    dma_engines[ei % len(dma_engines)].dma_start(
                        out=vtile[:, a * vw : b * vw], in_=vals_d[:, a * vw : b * vw]
                    )
                    ei += 1
            obuf = rpool.tile([16, gpc * ca], f32)

            # Scores + exp are per-token, so batch GEXP chunks (across slot
            # boundaries) into one PSUM bank and one ACT exp instruction.
            GEXP = 8
            attn_tiles = {}

            def ensure_attn(kk):
                gi = kk // GEXP
                if gi in attn_tiles:
                    return attn_tiles[gi]
                n = min(GEXP, n_chunks - gi * GEXP)
                sc = scpsum.tile([CHUNK, n * 16], f32, name=f"sc{gi}", tag="sc")
                for x in range(n):
                    t0 = (gi * GEXP + x) * CHUNK
                    nc.tensor.matmul(
                        sc[:, x * 16 : (x + 1) * 16],
                        ktile[:, t0 : t0 + CHUNK],
                        qt[:],
                        start=True,
                        stop=True,
                    )
                at = apool.tile([CHUNK, n * 16], f16, name=f"at{gi}", tag="a")
                nc.scalar.activation(at[:], sc[:], mybir.ActivationFunctionType.Exp)
                attn_tiles[gi] = at
                return at

            k = 0
            for j in range(gpc):
                nck = int(caps[j]) // CHUNK
                m = mpsum.tile([16, vw], f32, name=f"m{j}", tag="m")
                for ci in range(nck):
                    kk = k + ci
                    at = ensure_attn(kk)
                    x = kk % GEXP
                    nc.tensor.matmul(
                        m[:],
                        at[:, x * 16 : (x + 1) * 16],
                        vtile[:, kk * vw : (kk + 1) * vw],
                        start=(ci == 0),
                        stop=(ci == nck - 1),
                    )
                rec = recpool.tile([16, 1], f32, name=f"r{j}", tag="r")
                nc.vector.reciprocal(rec[:], m[:, ca : ca + 1])
                if j % 2 == 0:
                    nc.vector.tensor_scalar_mul(
                        obuf[:, j * ca : (j + 1) * ca], m[:, 0:ca], rec[:]
                    )
                else:
                    nc.scalar.activation(
                        obuf[:, j * ca : (j + 1) * ca],
                        m[:, 0:ca],
                        mybir.ActivationFunctionType.Copy,
                        scale=rec[:],
                    )
                k += nck

            out_v = out_d.rearrange("j b c -> b j c")
            obuf_v = obuf.rearrange("b (j c) -> b j c", c=ca)
            for i, (a, b) in enumerate(split(gpc, 4)):
                dma_engines[i % len(dma_engines)].dma_start(
                    out=out_v[:, a:b, :], in_=obuf_v[:, a:b, :]
                )

    nc.finalize()
    return nc


def kernel(Z_img, Z_snd, pad_idx, pad_mask, attn_dims):
    global LAST_RESULTS
    import os

    from concourse.bass_utils import run_bass_kernel_spmd

    Z_img = np.asarray(Z_img, dtype=np.float32)
    Z_snd = np.asarray(Z_snd, dtype=np.float32)
    pad_idx = np.asarray(pad_idx)
    pad_mask = np.asarray(pad_mask).astype(bool)
    A = int(attn_dims)

    B = Z_img.shape[0]
    C = Z_img.shape[1]
    CA = C - A
    G = pad_idx.shape[0]
    assert B == 16 and G % N_CORES == 0, (B, G)
    gpc = G // N_CORES

    z = Z_img.reshape(B, C, -1).mean(axis=2)
    z_img, query = z[:, :CA], z[:, CA:]

    sizes = pad_mask.sum(axis=1).astype(np.int64)
    order = np.argsort(-sizes, kind="stable")  # group ids, size descending
    caps = -(-np.maximum(sizes[order[0::N_CORES]], 1) // CHUNK) * CHUNK
    sum_caps = int(caps.sum())
    slot_off = np.concatenate([[0], np.cumsum(caps)[:-1]]).astype(np.int64)

    q_norm_max = float(np.linalg.norm(query, axis=1).max())

    # Per-core host-side layout prep.
    in_maps = []
    for c in range(N_CORES):
        keysT = np.zeros((65, sum_caps), dtype=np.float32)
        keysT[64, :] = -30000.0  # pad columns -> exp == 0 exactly
        vals = np.zeros((sum_caps, CA + 2), dtype=np.float32)
        for j in range(gpc):
            g = int(order[j * N_CORES + c])
            s = int(sizes[g])
            o = int(slot_off[j])
            if s == 0:
                # Reference yields NaN for empty groups (softmax of all
                # -inf); emit 0 instead via one fake zero-valued token.
                keysT[64, o] = 0.0
                vals[o, CA] = 1.0
                continue
            idx = pad_idx[g][pad_mask[g]]
            rows = Z_snd[idx]
            keysT[:64, o : o + s] = rows[:, CA:].T
            k_norm_max = float(np.linalg.norm(rows[:, CA:], axis=1).max())
            shift = min(q_norm_max * k_norm_max, 80.0)
            keysT[64, o : o + s] = -shift
            vals[o : o + s, :CA] = rows[:, :CA]
            vals[o : o + s, CA] = 1.0
        qT = np.empty((65, 16), dtype=np.float32)
        qT[:64] = query.T
        qT[64] = 1.0
        n_chunks = sum_caps // CHUNK
        vimg = np.ascontiguousarray(
            vals.reshape(n_chunks, CHUNK, CA + 2).transpose(1, 0, 2)
        ).reshape(CHUNK, n_chunks * (CA + 2)).astype(np.float16)
        in_maps.append({"keysT": keysT, "vals": vimg, "qT": qT})

    nc = _build_program(caps, gpc, CA)
    trace = bool(os.environ.get("AUDIOATTN_TRACE"))
    res = run_bass_kernel_spmd(
        nc, in_maps, list(range(N_CORES)), trace=trace,
        tmpdir=os.environ.get("AUDIOATTN_TRACE_DIR") if trace else None,
    )
    LAST_RESULTS = res

    M_snd = np.empty((G, B, CA), dtype=np.float32)
    for c in range(N_CORES):
        out_c = res.results[c]["out"]
        for j in range(gpc):
            M_snd[order[j * N_CORES + c]] = out_c[j]

    M_img = np.broadcast_to(z_img[None], (G, B, CA))
    return M_img, M_snd


# revision 12
# speedup vs baseline: 1.4060x; 1.0623x over previous
"""AudioAttention forward on 8 Trainium2 NeuronCores (Bass/Tile).

Reference computation (eval-mode AudioAttention):
    z      = mean_pool(Z_img)                    # [B, C]
    z_img, query = z[:, :C-A], z[:, C-A:]
    snd    = Z_snd[pad_idx]                      # [G, S, C] ragged gather
    value, key = snd[..., :C-A], snd[..., C-A:]
    scores = query @ key^T  (per group), masked softmax over S
    M_snd  = attn @ value                        # [G, B, C-A]
    M_img  = broadcast(z_img)                    # [G, B, C-A]

Sharding: groups are sorted by size and dealt round-robin to the 8
cores, so every core gets the same per-slot capacity profile -> one
SPMD program serves all cores (only the DRAM contents differ).
Capacities are rounded up to multiples of 128 so every 128-token chunk
is partition-aligned, which lets the whole key/value arrays live in
SBUF and stream in via a handful of large DMAs (the DMA *instruction*
dispatch on the queue engine costs ~0.8us each, so instruction count
matters more than bytes).

Device kernel, per 128-token chunk k of group slot j:
  scoresT [128,B] = matmul(lhsT=keyT_ext[65, 128], rhs=qT_ext[65, B])
      where row 64 of keyT_ext carries (-shift) for valid tokens and
      -30000 for padding, and row 64 of qT_ext is ones -> the mask and
      the softmax shift are folded into the contraction for free
      (exp(-30000) == 0 exactly, so pad tokens vanish).
  attnT = exp(scoresT)              (ACT engine, PSUM -> SBUF)
  m_j [B, 450] += matmul(lhsT=attnT[128, B], rhs=val_ext[128, 450])
      where val_ext column 448 is 1.0 for valid rows -> column 448
      accumulates the softmax denominator (449 is zero padding: the
      fp32r matmul requires an even destination free size).
  out_j = m_j[:, :448] * reciprocal(m_j[:, 448])  (DVE), one final DMA.

Matmuls run as float32r (TF32): same bits as fp32, 4x the fp32 PE
streaming rate. The softmax shift per group is a Cauchy-Schwarz upper
bound on the scores (max_b |q_b| * max_s |k_s|), so exp never
overflows; softmax is shift-invariant so the result is exact.
"""

import sys

if "/opt/trn_rl_repo" not in sys.path:
    sys.path.insert(0, "/opt/trn_rl_repo")

import numpy as np

N_CORES = 8
CHUNK = 128
N_SPLIT_V = 8  # value-array DMA split (parallel queues + early compute start)
N_SPLIT_K = 4  # key-array DMA split

LAST_RESULTS = None  # BassKernelResults of the most recent run (for test harness)


def _build_program(caps, gpc, ca):
    """One Bass program shared by all 8 cores.

    caps: per-slot token capacities, multiples of 128 (same on every core).
    gpc:  groups (slots) per core.
    ca:   C - A (value feature width).
    """
    from concourse import bacc, mybir
    from concourse.tile import TileContext

    vw = ca + 2  # value row width: features + denominator column + pad
    n_chunks = int(sum(caps)) // CHUNK
    sum_caps = n_chunks * CHUNK
    nc = bacc.Bacc(None, target_bir_lowering=False, debug=False)

    f32 = mybir.dt.float32
    f32r = mybir.dt.float32r
    f16 = mybir.dt.float16
    keys_d = nc.dram_tensor("keysT", [65, sum_caps], f32r, kind="ExternalInput")
    vals_d = nc.dram_tensor("vals", [CHUNK, n_chunks * vw], f16, kind="ExternalInput")
    qt_d = nc.dram_tensor("qT", [65, 16], f32r, kind="ExternalInput")
    out_d = nc.dram_tensor("out", [16, gpc * ca], f32, kind="ExternalOutput")


    def split(n, parts):
        q, r = divmod(n, parts)
        out, a = [], 0
        for i in range(parts):
            b = a + q + (1 if i < r else 0)
            if b > a:
                out.append((a, b))
            a = b
        return out

    with TileContext(nc) as tc:
        with (
            tc.tile_pool(name="resid", bufs=1) as rpool,
            tc.tile_pool(name="attn", bufs=3) as apool,
            tc.tile_pool(name="recp", bufs=4) as recpool,
            tc.tile_pool(name="scps", bufs=2, space="PSUM") as scpsum,
            tc.tile_pool(name="mps", bufs=4, space="PSUM") as mpsum,
            tc.tile_pool(name="wps", bufs=1, space="PSUM") as wpsum,
        ):
            qt = rpool.tile([65, 16], f32r)
            nc.sync.dma_start(out=qt[:], in_=qt_d[:])
            ktile = rpool.tile([65, sum_caps], f32r)
            vtile = rpool.tile([CHUNK, n_chunks * vw], f16)
            # Interleave key/value loads in consumption order, spread across
            # queue engines so the transfers ride parallel DMA channels.
            dma_engines = [nc.sync, nc.gpsimd]
            kparts = split(n_chunks, N_SPLIT_K)
            vparts = split(n_chunks, N_SPLIT_V)
            ei = 0
            for i in range(max(len(kparts), len(vparts))):
                if i < len(kparts):
                    a, b = kparts[i]
                    dma_engines[ei % len(dma_engines)].dma_start(
                        out=ktile[:, a * CHUNK : b * CHUNK],
                        in_=keys_d[:, a * CHUNK : b * CHUNK],
                    )
                    ei += 1
                if i < len(vparts):
                    a, b = vparts[i]
                    dma_engines[ei % len(dma_engines)].dma_start(
                        out=vtile[:, a * vw : b * vw], in_=vals_d[:, a * vw : b * vw]
                    )
                    ei += 1
            obuf = rpool.tile([16, gpc * ca], f32)

            bf16 = mybir.dt.bfloat16
            warm = rpool.tile([CHUNK, 512], bf16)
            nc.vector.memset(warm[:], 0.0)
            wps = wpsum.tile([CHUNK, 512], f32)
            for _ in range(10):
                nc.tensor.matmul(wps[:], warm[:, :CHUNK], warm[:], start=True, stop=True)

            # Scores + exp are per-token, so batch GEXP chunks (across slot
            # boundaries) into one PSUM bank and one ACT exp instruction.
            GEXP = 8
            attn_tiles = {}

            def ensure_attn(kk):
                gi = kk // GEXP
                if gi in attn_tiles:
                    return attn_tiles[gi]
                n = min(GEXP, n_chunks - gi * GEXP)
                sc = scpsum.tile([CHUNK, n * 16], f32, name=f"sc{gi}", tag="sc")
                for x in range(n):
                    t0 = (gi * GEXP + x) * CHUNK
                    nc.tensor.matmul(
                        sc[:, x * 16 : (x + 1) * 16],
                        ktile[:, t0 : t0 + CHUNK],
                        qt[:],
                        start=True,
                        stop=True,
                    )
                at = apool.tile([CHUNK, n * 16], f16, name=f"at{gi}", tag="a")
                nc.scalar.activation(at[:], sc[:], mybir.ActivationFunctionType.Exp)
                attn_tiles[gi] = at
                return at

            k = 0
            for j in range(gpc):
                nck = int(caps[j]) // CHUNK
                m = mpsum.tile([16, vw], f32, name=f"m{j}", tag="m")
                for ci in range(nck):
                    kk = k + ci
                    at = ensure_attn(kk)
                    x = kk % GEXP
                    nc.tensor.matmul(
                        m[:],
                        at[:, x * 16 : (x + 1) * 16],
                        vtile[:, kk * vw : (kk + 1) * vw],
                        start=(ci == 0),
                        stop=(ci == nck - 1),
                    )
                rec = recpool.tile([16, 1], f32, name=f"r{j}", tag="r")
                nc.vector.reciprocal(rec[:], m[:, ca : ca + 1])
                if j % 2 == 0:
                    nc.vector.tensor_scalar_mul(
                        obuf[:, j * ca : (j + 1) * ca], m[:, 0:ca], rec[:]
                    )
                else:
                    nc.scalar.activation(
                        obuf[:, j * ca : (j + 1) * ca],
                        m[:, 0:ca],
                        mybir.ActivationFunctionType.Copy,
                        scale=rec[:],
                    )
                k += nck

            for i, (a, b) in enumerate(split(gpc, 4)):
                dma_engines[i % len(dma_engines)].dma_start(
                    out=out_d[:, a * ca : b * ca], in_=obuf[:, a * ca : b * ca]
                )

    nc.finalize()
    return nc


def kernel(Z_img, Z_snd, pad_idx, pad_mask, attn_dims):
    global LAST_RESULTS
    import os

    from concourse.bass_utils import run_bass_kernel_spmd

    Z_img = np.asarray(Z_img, dtype=np.float32)
    Z_snd = np.asarray(Z_snd, dtype=np.float32)
    pad_idx = np.asarray(pad_idx)
    pad_mask = np.asarray(pad_mask).astype(bool)
    A = int(attn_dims)

    B = Z_img.shape[0]
    C = Z_img.shape[1]
    CA = C - A
    G = pad_idx.shape[0]
    assert B == 16 and G % N_CORES == 0, (B, G)
    gpc = G // N_CORES

    z = Z_img.reshape(B, C, -1).mean(axis=2)
    z_img, query = z[:, :CA], z[:, CA:]

    sizes = pad_mask.sum(axis=1).astype(np.int64)
    order = np.argsort(-sizes, kind="stable")  # group ids, size descending
    caps = -(-np.maximum(sizes[order[0::N_CORES]], 1) // CHUNK) * CHUNK
    sum_caps = int(caps.sum())
    slot_off = np.concatenate([[0], np.cumsum(caps)[:-1]]).astype(np.int64)

    q_norm_max = float(np.linalg.norm(query, axis=1).max())

    # Per-core host-side layout prep.
    in_maps = []
    for c in range(N_CORES):
        keysT = np.zeros((65, sum_caps), dtype=np.float32)
        keysT[64, :] = -30000.0  # pad columns -> exp == 0 exactly
        vals = np.zeros((sum_caps, CA + 2), dtype=np.float32)
        for j in range(gpc):
            g = int(order[j * N_CORES + c])
            s = int(sizes[g])
            o = int(slot_off[j])
            if s == 0:
                # Reference yields NaN for empty groups (softmax of all
                # -inf); emit 0 instead via one fake zero-valued token.
                keysT[64, o] = 0.0
                vals[o, CA] = 1.0
                continue
            idx = pad_idx[g][pad_mask[g]]
            rows = Z_snd[idx]
            keysT[:64, o : o + s] = rows[:, CA:].T
            k_norm_max = float(np.linalg.norm(rows[:, CA:], axis=1).max())
            shift = min(q_norm_max * k_norm_max, 80.0)
            keysT[64, o : o + s] = -shift
            vals[o : o + s, :CA] = rows[:, :CA]
            vals[o : o + s, CA] = 1.0
        qT = np.empty((65, 16), dtype=np.float32)
        qT[:64] = query.T
        qT[64] = 1.0
        n_chunks = sum_caps // CHUNK
        vimg = np.ascontiguousarray(
            vals.reshape(n_chunks, CHUNK, CA + 2).transpose(1, 0, 2)
        ).reshape(CHUNK, n_chunks * (CA + 2)).astype(np.float16)
        in_maps.append({"keysT": keysT, "vals": vimg, "qT": qT})

    nc = _build_program(caps, gpc, CA)
    trace = bool(os.environ.get("AUDIOATTN_TRACE"))
    res = run_bass_kernel_spmd(
        nc, in_maps, list(range(N_CORES)), trace=trace,
        tmpdir=os.environ.get("AUDIOATTN_TRACE_DIR") if trace else None,
    )
    LAST_RESULTS = res

    M_snd = np.empty((G, B, CA), dtype=np.float32)
    for c in range(N_CORES):
        out_c = res.results[c]["out"].reshape(B, gpc, CA)
        for j in range(gpc):
            M_snd[order[j * N_CORES + c]] = out_c[:, j]

    M_img = np.broadcast_to(z_img[None], (G, B, CA))
    return M_img, M_snd


# revision 13
# speedup vs baseline: 1.4789x; 1.0518x over previous
"""AudioAttention forward on 8 Trainium2 NeuronCores (Bass/Tile).

Reference computation (eval-mode AudioAttention):
    z      = mean_pool(Z_img)                    # [B, C]
    z_img, query = z[:, :C-A], z[:, C-A:]
    snd    = Z_snd[pad_idx]                      # [G, S, C] ragged gather
    value, key = snd[..., :C-A], snd[..., C-A:]
    scores = query @ key^T  (per group), masked softmax over S
    M_snd  = attn @ value                        # [G, B, C-A]
    M_img  = broadcast(z_img)                    # [G, B, C-A]

Sharding: groups are sorted by size and dealt round-robin to the 8
cores, so every core gets the same per-slot capacity profile -> one
SPMD program serves all cores (only the DRAM contents differ).
Capacities are rounded up to multiples of 128 so every 128-token chunk
is partition-aligned, which lets the whole key/value arrays live in
SBUF and stream in via a handful of large DMAs (the DMA *instruction*
dispatch on the queue engine costs ~0.8us each, so instruction count
matters more than bytes).

Device kernel, per 128-token chunk k of group slot j:
  scoresT [128,B] = matmul(lhsT=keyT_ext[65, 128], rhs=qT_ext[65, B])
      where row 64 of keyT_ext carries (-shift) for valid tokens and
      -30000 for padding, and row 64 of qT_ext is ones -> the mask and
      the softmax shift are folded into the contraction for free
      (exp(-30000) == 0 exactly, so pad tokens vanish).
  attnT = exp(scoresT)              (ACT engine, PSUM -> SBUF)
  m_j [B, 450] += matmul(lhsT=attnT[128, B], rhs=val_ext[128, 450])
      where val_ext column 448 is 1.0 for valid rows -> column 448
      accumulates the softmax denominator (449 is zero padding: the
      fp32r matmul requires an even destination free size).
  out_j = m_j[:, :448] * reciprocal(m_j[:, 448])  (DVE), one final DMA.

Matmuls run as float32r (TF32): same bits as fp32, 4x the fp32 PE
streaming rate. The softmax shift per group is a Cauchy-Schwarz upper
bound on the scores (max_b |q_b| * max_s |k_s|), so exp never
overflows; softmax is shift-invariant so the result is exact.
"""

import sys

if "/opt/trn_rl_repo" not in sys.path:
    sys.path.insert(0, "/opt/trn_rl_repo")

import numpy as np

N_CORES = 8
CHUNK = 128
N_SPLIT_V = 8  # value-array DMA split (parallel queues + early compute start)
N_SPLIT_K = 4  # key-array DMA split

LAST_RESULTS = None  # BassKernelResults of the most recent run (for test harness)


def _build_program(caps, gpc, ca):
    """One Bass program shared by all 8 cores.

    caps: per-slot token capacities, multiples of 128 (same on every core).
    gpc:  groups (slots) per core.
    ca:   C - A (value feature width).
    """
    from concourse import bacc, mybir
    from concourse.tile import TileContext

    vw = ca + 2  # value row width: features + denominator column + pad
    n_chunks = int(sum(caps)) // CHUNK
    sum_caps = n_chunks * CHUNK
    nc = bacc.Bacc(None, target_bir_lowering=False, debug=False)

    f32 = mybir.dt.float32
    f32r = mybir.dt.float32r
    f16 = mybir.dt.float16
    keys_d = nc.dram_tensor("keysT", [65, sum_caps], f16, kind="ExternalInput")
    vals_d = nc.dram_tensor("vals", [CHUNK, n_chunks * vw], f16, kind="ExternalInput")
    qt_d = nc.dram_tensor("qT", [65, 16], f16, kind="ExternalInput")
    out_d = nc.dram_tensor("out", [16, gpc * ca], f32, kind="ExternalOutput")


    def split(n, parts):
        q, r = divmod(n, parts)
        out, a = [], 0
        for i in range(parts):
            b = a + q + (1 if i < r else 0)
            if b > a:
                out.append((a, b))
            a = b
        return out

    with TileContext(nc) as tc:
        with (
            tc.tile_pool(name="resid", bufs=1) as rpool,
            tc.tile_pool(name="attn", bufs=3) as apool,
            tc.tile_pool(name="recp", bufs=4) as recpool,
            tc.tile_pool(name="scps", bufs=2, space="PSUM") as scpsum,
            tc.tile_pool(name="mps", bufs=4, space="PSUM") as mpsum,
            tc.tile_pool(name="wps", bufs=1, space="PSUM") as wpsum,
        ):
            qt = rpool.tile([65, 16], f16)
            nc.sync.dma_start(out=qt[:], in_=qt_d[:])
            ktile = rpool.tile([65, sum_caps], f16)
            vtile = rpool.tile([CHUNK, n_chunks * vw], f16)
            # Keys ride the Scalar HWDGE channel, values alternate between
            # Sync and GpSimd, so the transfers use parallel DMA channels.
            # First parts are small (one exp-batch worth) so the
            # scores->exp->value pipeline starts as early as possible.
            dma_engines = [nc.sync, nc.gpsimd]

            def head_parts(n, head, parts):
                out = [(0, min(head, n))]
                if n > head:
                    out += [(a + head, b + head) for a, b in split(n - head, parts)]
                return out

            for a, b in head_parts(n_chunks, 8, N_SPLIT_K - 1):
                nc.scalar.dma_start(
                    out=ktile[:, a * CHUNK : b * CHUNK],
                    in_=keys_d[:, a * CHUNK : b * CHUNK],
                )
            for i, (a, b) in enumerate(head_parts(n_chunks, 8, N_SPLIT_V - 1)):
                dma_engines[i % 2].dma_start(
                    out=vtile[:, a * vw : b * vw], in_=vals_d[:, a * vw : b * vw]
                )
            obuf = rpool.tile([16, gpc * ca], f32)

            bf16 = mybir.dt.bfloat16
            warm = rpool.tile([CHUNK, 512], bf16)
            nc.vector.memset(warm[:], 0.0)
            wps = wpsum.tile([CHUNK, 512], f32)
            for _ in range(10):
                nc.tensor.matmul(wps[:], warm[:, :CHUNK], warm[:], start=True, stop=True)

            # Scores + exp are per-token, so batch GEXP chunks (across slot
            # boundaries) into one PSUM bank and one ACT exp instruction.
            GEXP = 8
            attn_tiles = {}

            def ensure_attn(kk):
                gi = kk // GEXP
                if gi in attn_tiles:
                    return attn_tiles[gi]
                n = min(GEXP, n_chunks - gi * GEXP)
                sc = scpsum.tile([CHUNK, n * 16], f32, name=f"sc{gi}", tag="sc")
                for x in range(n):
                    t0 = (gi * GEXP + x) * CHUNK
                    nc.tensor.matmul(
                        sc[:, x * 16 : (x + 1) * 16],
                        ktile[:, t0 : t0 + CHUNK],
                        qt[:],
                        start=True,
                        stop=True,
                    )
                at = apool.tile([CHUNK, n * 16], f16, name=f"at{gi}", tag="a")
                nc.scalar.activation(at[:], sc[:], mybir.ActivationFunctionType.Exp)
                attn_tiles[gi] = at
                return at

            k = 0
            for j in range(gpc):
                nck = int(caps[j]) // CHUNK
                m = mpsum.tile([16, vw], f32, name=f"m{j}", tag="m")
                for ci in range(nck):
                    kk = k + ci
                    at = ensure_attn(kk)
                    x = kk % GEXP
                    nc.tensor.matmul(
                        m[:],
                        at[:, x * 16 : (x + 1) * 16],
                        vtile[:, kk * vw : (kk + 1) * vw],
                        start=(ci == 0),
                        stop=(ci == nck - 1),
                    )
                rec = recpool.tile([16, 1], f32, name=f"r{j}", tag="r")
                nc.vector.reciprocal(rec[:], m[:, ca : ca + 1])
                if j % 2 == 0:
                    nc.vector.tensor_scalar_mul(
                        obuf[:, j * ca : (j + 1) * ca], m[:, 0:ca], rec[:]
                    )
                else:
                    nc.scalar.activation(
                        obuf[:, j * ca : (j + 1) * ca],
                        m[:, 0:ca],
                        mybir.ActivationFunctionType.Copy,
                        scale=rec[:],
                    )
                k += nck

            for i, (a, b) in enumerate(split(gpc, 4)):
                dma_engines[i % len(dma_engines)].dma_start(
                    out=out_d[:, a * ca : b * ca], in_=obuf[:, a * ca : b * ca]
                )

    nc.finalize()
    return nc


def kernel(Z_img, Z_snd, pad_idx, pad_mask, attn_dims):
    global LAST_RESULTS
    import os

    from concourse.bass_utils import run_bass_kernel_spmd

    Z_img = np.asarray(Z_img, dtype=np.float32)
    Z_snd = np.asarray(Z_snd, dtype=np.float32)
    pad_idx = np.asarray(pad_idx)
    pad_mask = np.asarray(pad_mask).astype(bool)
    A = int(attn_dims)

    B = Z_img.shape[0]
    C = Z_img.shape[1]
    CA = C - A
    G = pad_idx.shape[0]
    assert B == 16 and G % N_CORES == 0, (B, G)
    gpc = G // N_CORES

    z = Z_img.reshape(B, C, -1).mean(axis=2)
    z_img, query = z[:, :CA], z[:, CA:]

    sizes = pad_mask.sum(axis=1).astype(np.int64)
    order = np.argsort(-sizes, kind="stable")  # group ids, size descending
    caps = -(-np.maximum(sizes[order[0::N_CORES]], 1) // CHUNK) * CHUNK
    sum_caps = int(caps.sum())
    slot_off = np.concatenate([[0], np.cumsum(caps)[:-1]]).astype(np.int64)

    q_norm_max = float(np.linalg.norm(query, axis=1).max())

    # Per-core host-side layout prep.
    in_maps = []
    for c in range(N_CORES):
        keysT = np.zeros((65, sum_caps), dtype=np.float32)
        keysT[64, :] = -30000.0  # pad columns -> exp == 0 exactly
        vals = np.zeros((sum_caps, CA + 2), dtype=np.float32)
        for j in range(gpc):
            g = int(order[j * N_CORES + c])
            s = int(sizes[g])
            o = int(slot_off[j])
            if s == 0:
                # Reference yields NaN for empty groups (softmax of all
                # -inf); emit 0 instead via one fake zero-valued token.
                keysT[64, o] = 0.0
                vals[o, CA] = 1.0
                continue
            idx = pad_idx[g][pad_mask[g]]
            rows = Z_snd[idx]
            keysT[:64, o : o + s] = rows[:, CA:].T
            k_norm_max = float(np.linalg.norm(rows[:, CA:], axis=1).max())
            shift = min(q_norm_max * k_norm_max, 80.0)
            keysT[64, o : o + s] = -shift
            vals[o : o + s, :CA] = rows[:, :CA]
            vals[o : o + s, CA] = 1.0
        qT = np.empty((65, 16), dtype=np.float32)
        qT[:64] = query.T
        qT[64] = 1.0
        n_chunks = sum_caps // CHUNK
        vimg = np.ascontiguousarray(
            vals.reshape(n_chunks, CHUNK, CA + 2).transpose(1, 0, 2)
        ).reshape(CHUNK, n_chunks * (CA + 2)).astype(np.float16)
        in_maps.append({"keysT": keysT.astype(np.float16), "vals": vimg, "qT": qT.astype(np.float16)})

    nc = _build_program(caps, gpc, CA)
    trace = bool(os.environ.get("AUDIOATTN_TRACE"))
    res = run_bass_kernel_spmd(
        nc, in_maps, list(range(N_CORES)), trace=trace,
        tmpdir=os.environ.get("AUDIOATTN_TRACE_DIR") if trace else None,
    )
    LAST_RESULTS = res

    M_snd = np.empty((G, B, CA), dtype=np.float32)
    for c in range(N_CORES):
        out_c = res.results[c]["out"].reshape(B, gpc, CA)
        for j in range(gpc):
            M_snd[order[j * N_CORES + c]] = out_c[:, j]

    M_img = np.broadcast_to(z_img[None], (G, B, CA))
    return M_img, M_snd


# revision 14
# speedup vs baseline: 1.5414x; 1.0423x over previous
"""AudioAttention forward on 8 Trainium2 NeuronCores (Bass/Tile).

Reference computation (eval-mode AudioAttention):
    z      = mean_pool(Z_img)                    # [B, C]
    z_img, query = z[:, :C-A], z[:, C-A:]
    snd    = Z_snd[pad_idx]                      # [G, S, C] ragged gather
    value, key = snd[..., :C-A], snd[..., C-A:]
    scores = query @ key^T  (per group), masked softmax over S
    M_snd  = attn @ value                        # [G, B, C-A]
    M_img  = broadcast(z_img)                    # [G, B, C-A]

Sharding: groups are sorted by size and dealt round-robin to the 8
cores, so every core gets the same per-slot capacity profile -> one
SPMD program serves all cores (only the DRAM contents differ).
Capacities are rounded up to multiples of 128 so every 128-token chunk
is partition-aligned, which lets the whole key/value arrays live in
SBUF and stream in via a handful of large DMAs (the DMA *instruction*
dispatch on the queue engine costs ~0.8us each, so instruction count
matters more than bytes).

Device kernel, per 128-token chunk k of group slot j:
  scoresT [128,B] = matmul(lhsT=keyT_ext[65, 128], rhs=qT_ext[65, B])
      where row 64 of keyT_ext carries (-shift) for valid tokens and
      -30000 for padding, and row 64 of qT_ext is ones -> the mask and
      the softmax shift are folded into the contraction for free
      (exp(-30000) == 0 exactly, so pad tokens vanish).
  attnT = exp(scoresT)              (ACT engine, PSUM -> SBUF)
  m_j [B, 450] += matmul(lhsT=attnT[128, B], rhs=val_ext[128, 450])
      where val_ext column 448 is 1.0 for valid rows -> column 448
      accumulates the softmax denominator (449 is zero padding: the
      fp32r matmul requires an even destination free size).
  out_j = m_j[:, :448] * reciprocal(m_j[:, 448])  (DVE), one final DMA.

Matmuls run as float32r (TF32): same bits as fp32, 4x the fp32 PE
streaming rate. The softmax shift per group is a Cauchy-Schwarz upper
bound on the scores (max_b |q_b| * max_s |k_s|), so exp never
overflows; softmax is shift-invariant so the result is exact.
"""

import sys

if "/opt/trn_rl_repo" not in sys.path:
    sys.path.insert(0, "/opt/trn_rl_repo")

import numpy as np

N_CORES = 8
CHUNK = 128
N_SPLIT_V = 8  # value-array DMA split (parallel queues + early compute start)
N_SPLIT_K = 4  # key-array DMA split

LAST_RESULTS = None  # BassKernelResults of the most recent run (for test harness)


def _build_program(caps, gpc, ca):
    """One Bass program shared by all 8 cores.

    caps: per-slot token capacities, multiples of 128 (same on every core).
    gpc:  groups (slots) per core.
    ca:   C - A (value feature width).
    """
    from concourse import bacc, mybir
    from concourse.tile import TileContext

    vw = ca + 2  # value row width: features + denominator column + pad
    n_chunks = int(sum(caps)) // CHUNK
    sum_caps = n_chunks * CHUNK
    nc = bacc.Bacc(None, target_bir_lowering=False, debug=False)

    f32 = mybir.dt.float32
    f32r = mybir.dt.float32r
    f16 = mybir.dt.float16
    keys_d = nc.dram_tensor("keysT", [65, sum_caps], f16, kind="ExternalInput")
    vals_d = nc.dram_tensor("vals", [CHUNK, n_chunks * vw], f16, kind="ExternalInput")
    qt_d = nc.dram_tensor("qT", [65, 16], f16, kind="ExternalInput")
    out_d = nc.dram_tensor("out", [16, gpc * ca], f16, kind="ExternalOutput")


    def split(n, parts):
        q, r = divmod(n, parts)
        out, a = [], 0
        for i in range(parts):
            b = a + q + (1 if i < r else 0)
            if b > a:
                out.append((a, b))
            a = b
        return out

    with TileContext(nc) as tc:
        with (
            tc.tile_pool(name="resid", bufs=1) as rpool,
            tc.tile_pool(name="attn", bufs=3) as apool,
            tc.tile_pool(name="recp", bufs=4) as recpool,
            tc.tile_pool(name="scps", bufs=2, space="PSUM") as scpsum,
            tc.tile_pool(name="mps", bufs=4, space="PSUM") as mpsum,
            tc.tile_pool(name="wps", bufs=1, space="PSUM") as wpsum,
        ):
            qt = rpool.tile([65, 16], f16)
            nc.sync.dma_start(out=qt[:], in_=qt_d[:])
            ktile = rpool.tile([65, sum_caps], f16)
            vtile = rpool.tile([CHUNK, n_chunks * vw], f16)
            # All keys load FIRST (small array; the scores->exp chain gates
            # the whole pipeline), then values stream behind them. Only the
            # two HWDGE channels (Sync + Scalar/ACT) carry DMAs: involving
    	    # GpSimd (SWDGE) costs a ~4us queue drain in the kernel tail.
            dma_engines = [nc.sync, nc.scalar]

            def head_parts(n, head, parts):
                out = [(0, min(head, n))]
                if n > head:
                    out += [(a + head, b + head) for a, b in split(n - head, parts)]
                return out

            for i, (a, b) in enumerate(split(n_chunks, N_SPLIT_K)):
                dma_engines[i % 2].dma_start(
                    out=ktile[:, a * CHUNK : b * CHUNK],
                    in_=keys_d[:, a * CHUNK : b * CHUNK],
                )
            for i, (a, b) in enumerate(head_parts(n_chunks, 8, N_SPLIT_V - 1)):
                dma_engines[i % 2].dma_start(
                    out=vtile[:, a * vw : b * vw], in_=vals_d[:, a * vw : b * vw]
                )
            obuf = rpool.tile([16, gpc * ca], f16)

            bf16 = mybir.dt.bfloat16
            warm = rpool.tile([CHUNK, 512], bf16)
            nc.vector.memset(warm[:], 0.0)
            wps = wpsum.tile([CHUNK, 512], f32)
            for _ in range(10):
                nc.tensor.matmul(wps[:], warm[:, :CHUNK], warm[:], start=True, stop=True)

            # Scores + exp are per-token, so batch GEXP chunks (across slot
            # boundaries) into one PSUM bank and one ACT exp instruction.
            GEXP = 8
            attn_tiles = {}

            def ensure_attn(kk):
                gi = kk // GEXP
                if gi in attn_tiles:
                    return attn_tiles[gi]
                n = min(GEXP, n_chunks - gi * GEXP)
                sc = scpsum.tile([CHUNK, n * 16], f32, name=f"sc{gi}", tag="sc")
                for x in range(n):
                    t0 = (gi * GEXP + x) * CHUNK
                    nc.tensor.matmul(
                        sc[:, x * 16 : (x + 1) * 16],
                        ktile[:, t0 : t0 + CHUNK],
                        qt[:],
                        start=True,
                        stop=True,
                    )
                at = apool.tile([CHUNK, n * 16], f16, name=f"at{gi}", tag="a")
                nc.scalar.activation(at[:], sc[:], mybir.ActivationFunctionType.Exp)
                attn_tiles[gi] = at
                return at

            k = 0
            for j in range(gpc):
                nck = int(caps[j]) // CHUNK
                m = mpsum.tile([16, vw], f32, name=f"m{j}", tag="m")
                for ci in range(nck):
                    kk = k + ci
                    at = ensure_attn(kk)
                    x = kk % GEXP
                    nc.tensor.matmul(
                        m[:],
                        at[:, x * 16 : (x + 1) * 16],
                        vtile[:, kk * vw : (kk + 1) * vw],
                        start=(ci == 0),
                        stop=(ci == nck - 1),
                    )
                rec = recpool.tile([16, 1], f32, name=f"r{j}", tag="r")
                nc.vector.reciprocal(rec[:], m[:, ca : ca + 1])
                if j % 2 == 0:
                    nc.vector.tensor_scalar_mul(
                        obuf[:, j * ca : (j + 1) * ca], m[:, 0:ca], rec[:]
                    )
                else:
                    nc.scalar.activation(
                        obuf[:, j * ca : (j + 1) * ca],
                        m[:, 0:ca],
                        mybir.ActivationFunctionType.Copy,
                        scale=rec[:],
                    )
                k += nck

            for i, (a, b) in enumerate(split(gpc, 4)):
                dma_engines[i % len(dma_engines)].dma_start(
                    out=out_d[:, a * ca : b * ca], in_=obuf[:, a * ca : b * ca]
                )

    nc.finalize()
    return nc


def kernel(Z_img, Z_snd, pad_idx, pad_mask, attn_dims):
    global LAST_RESULTS
    import os

    from concourse.bass_utils import run_bass_kernel_spmd

    Z_img = np.asarray(Z_img, dtype=np.float32)
    Z_snd = np.asarray(Z_snd, dtype=np.float32)
    pad_idx = np.asarray(pad_idx)
    pad_mask = np.asarray(pad_mask).astype(bool)
    A = int(attn_dims)

    B = Z_img.shape[0]
    C = Z_img.shape[1]
    CA = C - A
    G = pad_idx.shape[0]
    assert B == 16 and G % N_CORES == 0, (B, G)
    gpc = G // N_CORES

    z = Z_img.reshape(B, C, -1).mean(axis=2)
    z_img, query = z[:, :CA], z[:, CA:]

    sizes = pad_mask.sum(axis=1).astype(np.int64)
    order = np.argsort(-sizes, kind="stable")  # group ids, size descending
    caps = -(-np.maximum(sizes[order[0::N_CORES]], 1) // CHUNK) * CHUNK
    sum_caps = int(caps.sum())
    slot_off = np.concatenate([[0], np.cumsum(caps)[:-1]]).astype(np.int64)

    q_norm_max = float(np.linalg.norm(query, axis=1).max())

    # Per-core host-side layout prep.
    in_maps = []
    for c in range(N_CORES):
        keysT = np.zeros((65, sum_caps), dtype=np.float32)
        keysT[64, :] = -30000.0  # pad columns -> exp == 0 exactly
        vals = np.zeros((sum_caps, CA + 2), dtype=np.float32)
        for j in range(gpc):
            g = int(order[j * N_CORES + c])
            s = int(sizes[g])
            o = int(slot_off[j])
            if s == 0:
                # Reference yields NaN for empty groups (softmax of all
                # -inf); emit 0 instead via one fake zero-valued token.
                keysT[64, o] = 0.0
                vals[o, CA] = 1.0
                continue
            idx = pad_idx[g][pad_mask[g]]
            rows = Z_snd[idx]
            keysT[:64, o : o + s] = rows[:, CA:].T
            k_norm_max = float(np.linalg.norm(rows[:, CA:], axis=1).max())
            shift = min(q_norm_max * k_norm_max, 80.0)
            keysT[64, o : o + s] = -shift
            vals[o : o + s, :CA] = rows[:, :CA]
            vals[o : o + s, CA] = 1.0
        qT = np.empty((65, 16), dtype=np.float32)
        qT[:64] = query.T
        qT[64] = 1.0
        n_chunks = sum_caps // CHUNK
        vimg = np.ascontiguousarray(
            vals.reshape(n_chunks, CHUNK, CA + 2).transpose(1, 0, 2)
        ).reshape(CHUNK, n_chunks * (CA + 2)).astype(np.float16)
        in_maps.append({"keysT": keysT.astype(np.float16), "vals": vimg, "qT": qT.astype(np.float16)})

    nc = _build_program(caps, gpc, CA)
    trace = bool(os.environ.get("AUDIOATTN_TRACE"))
    res = run_bass_kernel_spmd(
        nc, in_maps, list(range(N_CORES)), trace=trace,
        tmpdir=os.environ.get("AUDIOATTN_TRACE_DIR") if trace else None,
    )
    LAST_RESULTS = res

    M_snd = np.empty((G, B, CA), dtype=np.float32)
    for c in range(N_CORES):
        out_c = res.results[c]["out"].astype(np.float32).reshape(B, gpc, CA)
        for j in range(gpc):
            M_snd[order[j * N_CORES + c]] = out_c[:, j]

    M_img = np.broadcast_to(z_img[None], (G, B, CA))
    return M_img, M_snd
